# revision 80
# baseline (speedup 1.0000x reference)
"""Trainium2 Bass kernel for nn_MultiHeadAttention_72378788872456 (v3).

Sharding (8 cores): core c handles batch b = c//4 and head group g = c%4
(heads 4g..4g+3).  Tensor-parallel on heads within each batch's 4-core
group; the only collective is a chunked 4-rank ReduceScatter after the
P_o contraction.

v2/v3 changes vs v1:
  - all inputs / weights / rope maps shipped as fp16 (half the HBM+DMA
    traffic); matmuls run fp16 (1 cycle/row, same PE rate as f32r).
  - sin/cos rope maps precomputed on host from the runtime positions
    (removes the on-device Cody-Waite sin pipeline and its DVE/ACT cost).
  - rope: psum->fp16 evacuation (ACT in prelude, DVE during attention),
    then fp16-SBUF DVE ops in 2x mode.
  - ReduceScatter wire in fp16 (halves collective payload).
  - deferred-work queue: output-projection pieces, the ReduceScatter and
    next q-chunk projections are emitted *inside* the next chunk's
    ACT-bound attention loop so the PE slack absorbs them (drained
    eagerly, one item per attention step).
  - tail shortening: psum o-accumulators evacuate to SBUF before the
    normalize chain (frees them for the next pair's matmuls), output
    staging is one wide tile + a single DMA per chunk, and the final
    drain splits staging copies across DVE and the then-idle ACT.

Layouts (per core):
  qT/kT/vT inputs: [128, DT, n] fp16   (contraction d on partitions)
  q/k after proj+rope: per head-pair tiles [128 = 2*64 k-dims, n] fp16
  scores S^T: [m, n] psum tiles; softmax denominator comes from a
  ones-column appended to the V stationary of the o^T matmul.
  o^T: [hv, n]; output projection emits natural [n, d] partials; 4-rank
  fp16 ReduceScatter per n-chunk.
"""

import math
import numpy as np

# ---------------------------------------------------------------- constants
B, N, M, D, H, K, V = 2, 2048, 2048, 1024, 16, 64, 64
MAX_WAVELENGTH = 10000.0
SCALE_FACTOR = 1.0
N_CORES = 8
HLOC = 4            # heads per core
PAIRS = HLOC // 2   # head-pairs per core
P = 128
FREE = 512          # matmul moving free-dim / n-chunk granularity

_COMPILED = {}


def build_nc(n=N, m=M, d=D, n_cores=N_CORES, group_size=4,
             use_collective=True, reps=1, shared_maps=True, phase="all",
             absorb=True, st16=True, rs16=True):
    """Build the SPMD Bass program (identical on every core)."""
    import concourse.bass as bass
    import concourse.mybir as mybir
    import concourse.tile as tile
    from concourse import bacc

    dt = mybir.dt
    f32 = dt.float32
    f32r = dt.float32r
    f16 = dt.float16
    AF = mybir.ActivationFunctionType
    ALU = mybir.AluOpType

    DT = d // P            # d tiles (contraction steps) for projections
    NC4 = n // FREE        # n chunks
    MT = m // P            # m tiles
    MC4 = m // FREE        # m chunks
    NTPC = FREE // P       # n tiles per chunk (outproj stationaries)
    DC = d // FREE         # d chunks in outproj output
    RG = [list(range(g * group_size, (g + 1) * group_size))
          for g in range(n_cores // group_size)]
    RS_ROWS = FREE // group_size  # 128

    nc = bacc.Bacc("TRN2", target_bir_lowering=False, debug=False,
                   num_devices=n_cores)

    # ------------------------------------------------ DRAM I/O declarations
    qT_d = nc.dram_tensor("qT", [P, DT, n], f16, kind="ExternalInput").ap()
    kT_d = nc.dram_tensor("kT", [P, DT, m], f16, kind="ExternalInput").ap()
    vT_d = nc.dram_tensor("vT", [P, DT, m], f16, kind="ExternalInput").ap()
    pq_d = nc.dram_tensor("pq", [P, DT, 2 * P], f16, kind="ExternalInput").ap()
    pk_d = nc.dram_tensor("pk", [P, DT, 2 * P], f16, kind="ExternalInput").ap()
    pv_d = nc.dram_tensor("pv", [P, DT, 2 * P], f16, kind="ExternalInput").ap()
    po_d = nc.dram_tensor("po", [P, PAIRS, d], f16, kind="ExternalInput").ap()
    # host-precomputed rope maps (sign-folded sin', cos), fp16
    sinq_d = nc.dram_tensor("sinq", [P, n], f16, kind="ExternalInput").ap()
    cosq_d = nc.dram_tensor("cosq", [P, n], f16, kind="ExternalInput").ap()
    if not shared_maps:
        sink_d = nc.dram_tensor("sink", [P, m], f16, kind="ExternalInput").ap()
        cosk_d = nc.dram_tensor("cosk", [P, m], f16, kind="ExternalInput").ap()
    ebc_d = nc.dram_tensor("ebc", [P, P], f32r, kind="ExternalInput").ap()
    vones_d = nc.dram_tensor("vones", [P, m // P, HLOC], f32r,
                             kind="ExternalInput").ap()
    fw = f16 if rs16 else f32       # collective wire dtype
    fst = f16 if st16 else f32r     # rope-output / S^T matmul dtype
    if use_collective:
        out_d = nc.dram_tensor("out_part", [NC4, RS_ROWS, d], fw,
                               kind="ExternalOutput").ap()
    else:
        out_d = nc.dram_tensor("out_part", [NC4, FREE, d], fw,
                               kind="ExternalOutput").ap()

    SWAP_MASK = [i ^ 1 for i in range(32)]

    with tile.TileContext(nc) as tc:
        with (
            tc.tile_pool(name="persist", bufs=1) as persist,
            tc.tile_pool(name="pmat", bufs=3) as pmatp,
            tc.tile_pool(name="instream", bufs=4) as instream,
            tc.tile_pool(name="ropep", bufs=3) as ropep,
            tc.tile_pool(name="expp", bufs=3) as expp,
            tc.tile_pool(name="nrm", bufs=2) as nrm,
            tc.tile_pool(name="otn", bufs=3) as otnp,
            tc.tile_pool(name="stps", bufs=2, space="PSUM") as stps,
            tc.tile_pool(name="otps", bufs=2, space="PSUM") as otps,
            tc.tile_pool(name="mmps", bufs=2, space="PSUM") as mmps,
            tc.tile_pool(name="dram", bufs=2, space="DRAM") as dramp,
        ):
            # persistent constants: DMA emission deferred into one_pass so
            # the first k-projection's inputs go out first.
            po_sb = persist.tile([P, PAIRS, d], f16, tag="po")
            ebc_sb = persist.tile([P, P], f32r, tag="ebc")
            zero_sb = persist.tile([P, FREE], f32, tag="zero")
            zr_sb = persist.tile([P, FREE], f32r, tag="zeror")
            nc.vector.memset(zero_sb[:], 0.0)
            nc.vector.tensor_copy(zr_sb[:], zero_sb[:])
            sinq_sb = persist.tile([P, n], f16, tag="sinq")
            cosq_sb = persist.tile([P, n], f16, tag="cosq")
            if shared_maps:
                sink_sb, cosk_sb = sinq_sb, cosq_sb
            else:
                sink_sb = persist.tile([P, m], f16, tag="sink")
                cosk_sb = persist.tile([P, m], f16, tag="cosk")

            def load_maps():
                nc.sync.dma_start(sinq_sb[:], sinq_d[:, :])
                nc.sync.dma_start(cosq_sb[:], cosq_d[:, :])
                if not shared_maps:
                    nc.sync.dma_start(sink_sb[:], sink_d[:, :])
                    nc.sync.dma_start(cosk_sb[:], cosk_d[:, :])

            def load_late():
                # needed only by normalize / outproj (~40 us in)
                nc.sync.dma_start(po_sb[:], po_d[:, :, :])
                nc.sync.dma_start(ebc_sb[:], ebc_d[:, :])

            def one_pass(rep):
                def project_rope_chunk(tin, p_sb, sin_sb, cos_sb, c, pair,
                                       name, evac="act"):
                    """One [P, FREE] rope'd fp16 projection tile."""
                    sl = slice(c * FREE, (c + 1) * FREE)
                    ps = mmps.tile([P, FREE], f32, tag="mm")
                    for t in range(DT):
                        nc.tensor.matmul(
                            ps[:], p_sb[:, t, pair * P:(pair + 1) * P],
                            tin[:, t, :],
                            start=(t == 0), stop=(t == DT - 1))
                    a16 = ropep.tile([P, FREE], f16, tag="a16")
                    if evac == "act":
                        nc.scalar.copy(a16[:], ps[:])
                    else:
                        nc.vector.tensor_copy(a16[:], ps[:])
                    xsw = ropep.tile([P, FREE], f16, tag="xsw")
                    nc.vector.stream_shuffle(xsw[:], a16[:], SWAP_MASK)
                    t1 = ropep.tile([P, FREE], f16, tag="t1")
                    nc.vector.tensor_tensor(t1[:], a16[:], cos_sb[:, sl],
                                            ALU.mult)
                    u = ropep.tile([P, FREE], f16, tag="u")
                    nc.vector.tensor_tensor(u[:], xsw[:], sin_sb[:, sl],
                                            ALU.mult)
                    out = persist.tile([P, FREE], fst, tag=f"rope_{name}",
                                       name=f"r{rep}_{name}")
                    with nc.allow_low_precision(reason="rope out"):
                        nc.vector.tensor_tensor(out[:], t1[:], u[:], ALU.add)
                    return out

                # ---------------- k + v (interleaved chunks)
                pk_sb = pmatp.tile([P, DT, 2 * P], f16, tag="pmat",
                                   name=f"pm_k{rep}")
                nc.sync.dma_start(pk_sb[:], pk_d[:, :, :])
                pv_sb = pmatp.tile([P, DT, 2 * P], f16, tag="pmat",
                                   name=f"pm_v{rep}")
                vsb = persist.tile([P, MT, HLOC * 65], f32r, tag="vsb",
                                   name=f"vsb{rep}")

                krope = [[None] * MC4 for _ in range(PAIRS)]

                def emit_k_chunk(c, evac="act", tin=None):
                    if tin is None:
                        tin = instream.tile([P, DT, FREE], f16, tag="instream")
                        nc.sync.dma_start(tin[:],
                                          kT_d[:, :, c * FREE:(c + 1) * FREE])
                    for pr in range(PAIRS):
                        krope[pr][c] = project_rope_chunk(
                            tin, pk_sb, sink_sb, cosk_sb, c, pr,
                            f"k{pr}_{c}", evac=evac)

                def emit_v_chunk(c, evac="act"):
                    tin = instream.tile([P, DT, FREE], f16, tag="instream")
                    nc.sync.dma_start(tin[:], vT_d[:, :, c * FREE:(c + 1) * FREE])
                    for mi4 in range(FREE // P):
                        mi = c * (FREE // P) + mi4
                        ps = mmps.tile([P, FREE], f32, tag="mm")
                        for t in range(DT):
                            nc.tensor.matmul(
                                ps[:, 0:2 * P], tin[:, t, mi4 * P:(mi4 + 1) * P],
                                pv_sb[:, t, :],
                                start=(t == 0), stop=(t == DT - 1))
                        dstv = vsb[:, mi, :].rearrange("p (h w) -> p h w", h=HLOC)[:, :, 0:64]
                        srcv = ps[:, 0:2 * P].rearrange("p (h w) -> p h w", h=HLOC)
                        if evac == "act":
                            nc.scalar.copy(dstv, srcv)
                        else:
                            nc.vector.tensor_copy(dstv, srcv)

                # minimal prelude: k0 / v0 / q0 only; later chunks are
                # emitted inside attention chunk 0's ACT-bound window.
                # DMA order: k0/q0's inputs first (unblocks the first
                # matmuls), rope maps next (needed by the first rope),
                # po/ebc last (needed ~40 us in).
                ABSORB = (phase == "all") and absorb
                tin_k0 = instream.tile([P, DT, FREE], f16, tag="instream")
                nc.sync.dma_start(tin_k0[:], kT_d[:, :, 0:FREE])
                load_maps()
                emit_k_chunk(0, tin=tin_k0)
                nc.sync.dma_start(pv_sb[:], pv_d[:, :, :])
                nc.sync.dma_start(
                    vsb[:].rearrange("p m (h w) -> p m h w", h=HLOC)[:, :, :, 64:65],
                    vones_d[:, :, :, None])
                emit_v_chunk(0)
                load_late()
                pq_sb = pmatp.tile([P, DT, 2 * P], f16, tag="pmat",
                                   name=f"pm_q{rep}")
                nc.sync.dma_start(pq_sb[:], pq_d[:, :, :])
                tin_q0 = instream.tile([P, DT, FREE], f16, tag="instream")
                nc.sync.dma_start(tin_q0[:], qT_d[:, :, 0:FREE])
                if not ABSORB:
                    for c in range(1, MC4):
                        emit_k_chunk(c)
                        emit_v_chunk(c)

                if phase == "proj":
                    cons = mmps.tile([P, FREE], f32, tag="mm")
                    toks = [krope[pr][c] for pr in range(PAIRS) for c in range(MC4)]
                    for i, tk in enumerate(toks):
                        nc.tensor.matmul(cons[:], tk[:, 0:P], tk[:],
                                         start=(i == 0), stop=(i == len(toks) - 1))
                    cons2 = mmps.tile([P, FREE], f32, tag="mm")
                    nc.tensor.matmul(cons2[:, 0:P], vsb[:, 0, 0:P],
                                     vsb[:, 1, 0:P],
                                     start=True, stop=True)
                    stgx = ropep.tile([P, FREE], f16, tag="stgx")
                    nc.vector.tensor_copy(stgx[:], cons[:])
                    nc.vector.tensor_copy(stgx[:, 0:P], cons2[:, 0:P])
                    nc.sync.dma_start(out_d[0, 0:P, 0:FREE], stgx[:, 0:FREE])
                    return

                # ---------------- q projection (pipelined with attention)
                qrope = [[None] * NC4 for _ in range(PAIRS)]

                qtins = {0: tin_q0}

                def emit_q_pair(c, pr, evac="act"):
                    if c not in qtins:
                        tin = instream.tile([P, DT, FREE], f16, tag="instream")
                        nc.sync.dma_start(tin[:],
                                          qT_d[:, :, c * FREE:(c + 1) * FREE])
                        qtins[c] = tin
                    qrope[pr][c] = project_rope_chunk(
                        qtins[c], pq_sb, sinq_sb, cosq_sb, c, pr,
                        f"q{pr}_{c}", evac=evac)

                def emit_q_chunk(c, evac="act"):
                    for pr in range(PAIRS):
                        emit_q_pair(c, pr, evac=evac)

                emit_q_chunk(0)
                if not ABSORB:
                    emit_q_chunk(1)

                # Deferred PE-side work, drained inside the ACT-bound
                # attention loops so PE slack absorbs it.  In the tail
                # (after the last exp) ACT is idle, so staging copies
                # alternate DVE/ACT there to shorten the critical chain.
                deferred = []
                tail_mode = [False]

                def drain(k=1):
                    for _ in range(k):
                        if deferred:
                            deferred.pop(0)()

                def push_outproj(cc, otns):
                    inb = dramp.tile([FREE, d], fw, tag="inb",
                                     name=f"inb{rep}_{cc}")
                    # two staging half-tiles -> two DMAs: fewer HWDGE
                    # overheads than 8 DMAs, but the first half's DMA can
                    # stream while the second half's copies still run
                    # (separate tiles keep the dependencies precise;
                    # dependency tracking is whole-tile)
                    stg_half = [
                        ropep.tile([P, NTPC * DC // 2, FREE], fw,
                                   tag=f"stgh{h}",
                                   name=f"stg{h}_{rep}_{cc}")
                        for h in range(2)
                    ]

                    def piece(nt, dc):
                        def go():
                            ops_ = mmps.tile([P, FREE], f32, tag="mm")
                            for t in range(PAIRS):
                                nc.tensor.matmul(
                                    ops_[:], otns[t][:, nt * P:(nt + 1) * P],
                                    po_sb[:, t, dc * FREE:(dc + 1) * FREE],
                                    start=(t == 0), stop=(t == PAIRS - 1))
                            idx = nt * DC + dc
                            half, slot = divmod(idx, NTPC * DC // 2)
                            dst = stg_half[half][:, slot, :]
                            if tail_mode[0] and (nt + dc) % 2 == 0:
                                nc.scalar.copy(dst, ops_[:])
                            else:
                                nc.vector.tensor_copy(dst, ops_[:])
                            if tail_mode[0] and idx == NTPC * DC // 2 - 1:
                                # first half complete: stream it now
                                emit_half_dma(0)
                        return go

                    for nt in range(NTPC):
                        for dc in range(DC):
                            deferred.append(piece(nt, dc))

                    HNT = NTPC // 2
                    dma_done = [False, False]

                    def emit_half_dma(h):
                        if dma_done[h]:
                            return
                        dma_done[h] = True
                        rows = slice(h * HNT * P, (h + 1) * HNT * P)
                        nc.sync.dma_start(
                            inb[rows, :].rearrange(
                                "(nt p) (dc f) -> p nt dc f",
                                nt=HNT, p=P, dc=DC, f=FREE),
                            stg_half[h][:].rearrange(
                                "p (nt dc) f -> p nt dc f",
                                nt=HNT, dc=DC))

                    def finish():
                        emit_half_dma(0)
                        emit_half_dma(1)
                        if use_collective:
                            outb = dramp.tile([RS_ROWS, d], fw, tag="outb",
                                              name=f"outb{rep}_{cc}")
                            nc.gpsimd.collective_compute(
                                "ReduceScatter", mybir.AluOpType.add,
                                replica_groups=RG,
                                ins=[inb.opt()], outs=[outb.opt()])
                            nc.sync.dma_start(out_d[cc, :, :], outb[:])
                        else:
                            nc.sync.dma_start(out_d[cc, :, :], inb[:])
                    deferred.append(finish)

                def emit_st(c, pr, mi):
                    stp = stps.tile([P, 2 * FREE], f32, tag="st",
                                    name=f"st{rep}_{c}_{pr}_{mi}")
                    for h in range(2):
                        hp = h * 64
                        nc.tensor.matmul(
                            stp[:, h * FREE:(h + 1) * FREE],
                            krope[pr][mi // (FREE // P)]
                                 [hp:hp + 64,
                                  (mi % (FREE // P)) * P:
                                  (mi % (FREE // P) + 1) * P],
                            qrope[pr][c][hp:hp + 64, :],
                            start=True, stop=True,
                            tile_position=(hp, 0))
                    return stp

                def normalize(c, pr, pot, last=False):
                    # evacuate pot to SBUF first so the PSUM accumulators
                    # free early (the next pair's o^T matmuls reuse them);
                    # skipped for the very last pair (nothing reuses them
                    # and it lengthens the tail's critical chain).
                    rrf = nrm.tile([P, FREE], f32, tag="rrf")
                    nc.vector.reciprocal(rrf[0:1, :], pot[0][64:65, :])
                    nc.vector.reciprocal(rrf[32:33, :], pot[1][64:65, :])
                    if not last:
                        pcp = nrm.tile([P, FREE], f32, tag="pcp")
                        nc.vector.tensor_copy(pcp[0:64, :], pot[0][0:64, :])
                        nc.vector.tensor_copy(pcp[64:128, :], pot[1][0:64, :])
                        srcs = (pcp[0:64, :], pcp[64:128, :])
                    else:
                        srcs = (pot[0][0:64, :], pot[1][0:64, :])
                    rz = nrm.tile([P, FREE], f32r, tag="rz")
                    nc.vector.tensor_copy(rz[:], zr_sb[:])
                    nc.vector.tensor_copy(rz[0:1, :], rrf[0:1, :])
                    nc.vector.tensor_copy(rz[32:33, :], rrf[32:33, :])
                    rb = mmps.tile([P, FREE], f32, tag="mm")
                    nc.tensor.matmul(rb[:], ebc_sb[:], rz[:],
                                     start=True, stop=True)
                    rbs = nrm.tile([P, FREE], f32, tag="rbs")
                    nc.vector.tensor_copy(rbs[:], rb[:])
                    ot = otnp.tile([P, FREE], f16, tag="otn")
                    nc.vector.tensor_tensor(ot[0:64, :], srcs[0],
                                            rbs[0:64, :], ALU.mult)
                    nc.vector.tensor_tensor(ot[64:128, :], srcs[1],
                                            rbs[64:128, :], ALU.mult)
                    return ot

                # one flat, software-pipelined stream over (chunk, pair, mi):
                # the next step's S^T matmuls are always in the PE queue
                # before the current exp, across pair and chunk boundaries.
                flat = [(c, pr, mi) for c in range(NC4)
                        for pr in range(PAIRS) for mi in range(MT)]
                stp_next = emit_st(*flat[0])
                pot = None
                otn_tiles = []
                for i, (c, pr, mi) in enumerate(flat):
                    stp = stp_next
                    stp_next = emit_st(*flat[i + 1]) if i + 1 < len(flat) else None
                    if mi == 0:
                        pot = [otps.tile([65, FREE], f32, tag="ot",
                                         name=f"ot{rep}_{c}_{pr}_{hh}")
                               for hh in range(2)]
                    ex = expp.tile([P, 2 * FREE], f32r, tag="exp")
                    nc.scalar.activation(ex[:], stp[:], AF.Exp)
                    for h in range(2):
                        hc = (2 * pr + h) * 65
                        nc.tensor.matmul(
                            pot[h][:], vsb[:, mi, hc:hc + 65],
                            ex[:, h * FREE:(h + 1) * FREE],
                            start=(mi == 0), stop=(mi == MT - 1))
                    # k_cm must be emitted by step 4*cm-2 (ST lookahead),
                    # v_cm by step 4*cm-1 (oT); spread to the latest slots.
                    KV_SLOTS = {0: ("k", 1), 3: ("v", 1), 6: ("k", 2),
                                7: ("v", 2), 10: ("k", 3), 11: ("v", 3)}
                    if c == 0 and pr == 0 and mi in KV_SLOTS and ABSORB:
                        kind, cm = KV_SLOTS[mi]
                        (emit_k_chunk if kind == "k" else emit_v_chunk)(
                            cm, evac="dve")
                    elif c == 0 and pr == 1 and mi in (0, 2) and ABSORB:
                        emit_q_pair(1, 0 if mi == 0 else 1, evac="dve")
                    elif pr == 1 and mi == 6 and c + 2 < NC4:
                        emit_q_pair(c + 2, 0, evac="dve")
                    elif pr == 1 and mi == 12 and c + 2 < NC4:
                        emit_q_pair(c + 2, 1, evac="dve")
                    elif mi < MT - 1:
                        drain(1)
                    if mi == MT - 1:
                        if phase == "attn":
                            stgx = ropep.tile([P, FREE], f16, tag="stgx")
                            nc.vector.tensor_copy(stgx[0:64, :], pot[0][0:64, :])
                            nc.vector.tensor_copy(stgx[64:128, :], pot[1][0:64, :])
                            nc.sync.dma_start(out_d[c, 0:P, 0:FREE], stgx[:])
                            continue
                        otn_tiles.append(normalize(
                            c, pr, pot,
                            last=(c == NC4 - 1 and pr == PAIRS - 1)))
                        if pr == PAIRS - 1:
                            push_outproj(c, otn_tiles)
                            otn_tiles = []
                # drain whatever is left (last chunk's outproj + RS);
                # ACT is idle now, share the staging copies with it
                tail_mode[0] = True
                drain(len(deferred))

            for rep in range(reps):
                if rep:
                    tc.strict_bb_all_engine_barrier()
                one_pass(rep)

    nc.compile()
    return nc


# ------------------------------------------------------------------- host

def _prep_core_inputs(query, q_positions, key, k_positions, value,
                      P_q, P_k, P_v, P_o, core, n=N, m=M, d=D,
                      shared_maps=True):
    """Per-core input map (numpy, host-side shard/layout prep, fp16)."""
    b = core // 4
    g = core % 4
    DT = d // P
    hsl = slice(g * HLOC, (g + 1) * HLOC)

    def t_in(x, length):  # [length, d] f32 -> [P, DT, length] fp16
        return np.ascontiguousarray(
            x.T.reshape(DT, P, length).transpose(1, 0, 2)).astype(np.float16)

    # interleaved k-dim order: stationary col c (per head) holds original
    # k index (c%2)*32 + c//2, so the rope partner sits on the adjacent
    # partition (stream_shuffle-able swap).
    KPERM = np.array([(c % 2) * 32 + c // 2 for c in range(64)])

    def pack_pqk(Pm):  # [HLOC, d, 64] -> [P, DT, 2*P] fp16
        out = np.empty((P, DT, 2 * P), np.float16)
        for p in range(PAIRS):
            for hl in range(2):
                h = 2 * p + hl
                out[:, :, p * P + hl * 64: p * P + hl * 64 + 64] = \
                    Pm[h].reshape(DT, P, 64).transpose(1, 0, 2)[:, :, KPERM]
        return np.ascontiguousarray(out)

    def pack_pv(Pm):  # [HLOC, d, 64] -> [P, DT, 256] fp16
        return np.ascontiguousarray(
            Pm.reshape(HLOC, DT, P, 64).transpose(2, 1, 0, 3)
            .reshape(P, DT, 2 * P)).astype(np.float16)

    def pack_po(Pm):  # [HLOC, d, V] -> [P, PAIRS, d] fp16
        out = np.empty((P, PAIRS, d), np.float16)
        for t in range(PAIRS):
            for hl in range(2):
                h = 2 * t + hl
                out[hl * 64:(hl + 1) * 64, t, :] = Pm[h].T
        return np.ascontiguousarray(out)

    # rope maps: partition row j holds interleaved k-dim (j%64):
    # j_idx = (j%64)//2 timescale index, half = j%2 (0 -> x1 row: -sin).
    jj = np.arange(P) % 64
    j_idx = jj // 2
    half = jj % 2
    frac = 2.0 * j_idx.astype(np.float64) / 64.0
    invt = (np.float64(MAX_WAVELENGTH) ** (-frac)) / np.float64(SCALE_FACTOR)
    sign = np.where(half == 0, -1.0, 1.0)

    def maps(pos):  # [length] int -> sin'[P, length], cos[P, length] fp16
        ph = invt[:, None] * pos[None, :].astype(np.float64)
        return ((sign[:, None] * np.sin(ph)).astype(np.float16),
                np.cos(ph).astype(np.float16))

    sinq, cosq = maps(np.asarray(q_positions[b]))

    ebc = np.zeros((P, P), np.float32)
    ebc[0, 0:64] = 1.0
    ebc[32, 64:128] = 1.0
    vones = np.ones((P, m // P, HLOC), np.float32)

    inp = {
        "qT": t_in(query[b], n),
        "kT": t_in(key[b], m),
        "vT": t_in(value[b], m),
        "pq": pack_pqk(P_q[hsl]),
        "pk": pack_pqk(P_k[hsl]),
        "pv": pack_pv(P_v[hsl]),
        "po": pack_po(P_o[hsl]),
        "sinq": sinq,
        "cosq": cosq,
        "ebc": ebc,
        "vones": vones,
    }
    if not shared_maps:
        sink, cosk = maps(np.asarray(k_positions[b]))
        inp["sink"] = sink
        inp["cosk"] = cosk
    return inp


def assemble_output(results, n=N, d=D, group_size=4):
    """Gather per-core [NC4, 128, d] fp16 RS shards into the full [B, n, d]."""
    NC4 = n // FREE
    rows = FREE // group_size
    out = np.empty((B, n, d), np.float32)
    for core in range(N_CORES):
        b, r = core // group_size, core % group_size
        part = np.asarray(results[core]["out_part"]).astype(np.float32)
        part = part.reshape(NC4, rows, d)
        for c in range(NC4):
            out[b, c * FREE + r * rows:(c * FREE + (r + 1) * rows), :] = part[c]
    return out


def kernel(query, q_positions, key, k_positions, value, mask=None,
           P_q=None, P_k=None, P_v=None, P_o=None, **_unused):
    from concourse.bass_utils import run_bass_kernel_spmd

    query = np.asarray(query, np.float32)
    key = np.asarray(key, np.float32)
    value = np.asarray(value, np.float32)
    q_positions = np.asarray(q_positions, np.int32)
    k_positions = np.asarray(k_positions, np.int32)
    P_q = np.asarray(P_q, np.float32)
    P_k = np.asarray(P_k, np.float32)
    P_v = np.asarray(P_v, np.float32)
    P_o = np.asarray(P_o, np.float32)

    shared = (N == M) and np.array_equal(q_positions, k_positions)
    key_dims = (N, M, D, shared)
    if key_dims not in _COMPILED:
        _COMPILED[key_dims] = build_nc(N, M, D, shared_maps=shared)
    nc = _COMPILED[key_dims]

    in_maps = [
        _prep_core_inputs(query, q_positions, key, k_positions, value,
                          P_q, P_k, P_v, P_o, core, shared_maps=shared)
        for core in range(N_CORES)
    ]
    res = run_bass_kernel_spmd(nc, in_maps, list(range(N_CORES)))
    return assemble_output(res.results)


if __name__ == "__main__":
    print("building...")
    build_nc()
    print("ok")
